# revision 2
# baseline (speedup 1.0000x reference)
"""Trainium2 Bass kernel for the rumor-GCN masked-autoencoder loss.

Strategy (8 NeuronCores, SPMD single NEFF):
  - Nodes are partitioned into 8 contiguous ranges (25000 each). Each core
    aggregates the in-edges of its own nodes (TD: grouped by dst, BU: grouped
    by src) -- "halo exchange" is done by the host pre-gathering the x-rows
    each core needs, so layer-1 is fully local.
  - Layer-1 linear is one fused [512->512] bf16 matmul over each core's
    needed-node set (4 GCN heads side by side). GCN symmetric norms are
    folded into per-row scales (dinv[src] into z at the P1 copy-out, dinv[dst]
    at aggregation finalize), so the sparse selection matrices are pure 0/1.
  - Edge aggregation: dma_gather pulls source rows into K-tiles of 128 edge
    slots; a one-hot S matrix per K-tile turns scatter-add into PE matmuls
    accumulating in PSUM per 128-dst-node block.
  - Layer-2 inputs (z2) are AllGathered across cores, then the same gather/
    matmul machinery runs against global 32K-row windows (int16 gather idx).
  - global_add_pool via matmuls into a persistent PSUM accumulator; the
    pooled sums + the masked-node cosine partial sum go through one small
    AllReduce; every core then computes the identical final scalar loss.
"""

import sys

import numpy as np

sys.path.insert(0, "/opt/trn_rl_repo")

# ---------------------------------------------------------------- config

class Cfg:
    def __init__(self, N, E, G, M, C=8, WIN=32768, GB=16, GB2=8, NF=2048):
        self.N, self.E, self.G, self.M, self.C = N, E, G, M, C
        self.IN, self.HID, self.OUT = 512, 128, 64
        self.WIN, self.GB, self.GB2, self.NF = WIN, GB, GB2, NF
        assert N % C == 0
        self.OWN = N // C
        self.NB = -(-self.OWN // 128)
        self.OWNP = self.NB * 128
        self.NPAD = C * self.OWNP
        self.NW2 = -(-self.NPAD // WIN)


FULL = Cfg(N=200000, E=400000, G=128, M=100000)

_WNAMES = [p + s for p in ("on_td", "on_bu", "tgt_td", "tgt_bu")
           for s in ("_W1", "_b1", "_W2", "_b2")]


def _rep16(idx_flat, nslots):
    """int16 index list -> [128, nslots//16] layout (16-part wrap, 8x replicated)."""
    blk = np.zeros((16, nslots // 16), dtype=np.int16)
    k = np.arange(len(idx_flat))
    blk[k % 16, k // 16] = idx_flat
    return np.tile(blk, (8, 1))


def _bcast(vec, parts=128):
    return np.broadcast_to(np.asarray(vec)[None, :], (parts, len(vec))).copy()


# ---------------------------------------------------------------- host prep

def host_prep(cfg, inp):
    import ml_dtypes
    bf16 = ml_dtypes.bfloat16
    c = cfg
    x = np.asarray(inp["x"], np.float32)
    token = np.asarray(inp["enc_mask_token"], np.float32).reshape(-1)
    ei = np.asarray(inp["edge_index"])
    src, dst = ei[0].astype(np.int64), ei[1].astype(np.int64)
    batch = np.asarray(inp["batch"]).astype(np.int64)
    mask_nodes = np.asarray(inp["mask_nodes"]).astype(np.int64)
    W = {k: np.asarray(inp[k], np.float32) for k in _WNAMES}

    dinv_td = (1.0 / np.sqrt(np.bincount(dst, minlength=c.N) + 1.0)).astype(np.float32)
    dinv_bu = (1.0 / np.sqrt(np.bincount(src, minlength=c.N) + 1.0)).astype(np.float32)
    is_masked = np.zeros(c.N, bool)
    is_masked[mask_nodes] = True
    xbf = x.astype(bf16)

    # ---- per-core edge lists (agg_dst local block/lane, agg_src global)
    # dir 0 = TD (aggregate src -> dst, dinv_td); dir 1 = BU (dst -> src, dinv_bu)
    core_edges = []   # [core][dir] -> (adst_local, asrc_global)
    for ci in range(c.C):
        lo, hi = ci * c.OWN, (ci + 1) * c.OWN
        per = []
        for d in range(2):
            ad, as_ = (dst, src) if d == 0 else (src, dst)
            sel = (ad >= lo) & (ad < hi)
            adst = ad[sel] - lo
            asrc = as_[sel]
            # self loops
            adst = np.concatenate([adst, np.arange(c.OWN, dtype=np.int64)])
            asrc = np.concatenate([asrc, np.arange(lo, hi, dtype=np.int64)])
            per.append((adst, asrc))
        core_edges.append(per)

    # ---- needed sets / z-row maps
    RU, RK = [], []
    needed_um, needed_mk, localmap = [], [], []
    for ci in range(c.C):
        lo, hi = ci * c.OWN, (ci + 1) * c.OWN
        nodes = np.unique(np.concatenate(
            [np.arange(lo, hi, dtype=np.int64),
             core_edges[ci][0][1], core_edges[ci][1][1]]))
        um = nodes[~is_masked[nodes]]
        mk = nodes[is_masked[nodes]]
        needed_um.append(um); needed_mk.append(mk)
        RU.append(len(um)); RK.append(len(mk))
    RU_PAD = -(-max(RU) // 128) * 128
    RK_PAD = -(-max(RK) // 128) * 128
    RT = RU_PAD + RK_PAD
    NW1 = -(-RT // c.WIN)
    for ci in range(c.C):
        lm = np.full(c.N, -1, np.int64)
        lm[needed_um[ci]] = np.arange(RU[ci])
        lm[needed_mk[ci]] = RU_PAD + np.arange(RK[ci])
        localmap.append(lm)

    # ---- slot schedules: for (dir, layer) build common KT[b][w], flat kt order
    # flat order: group g -> window w -> block b in g -> tiles
    def build_sched(layer):
        GB = c.GB if layer == 0 else c.GB2
        NG = -(-c.NB // GB)
        scheds = []
        for d in range(2):
            NW = NW1 if layer == 0 else c.NW2
            cnt = np.zeros((c.C, c.NB, NW), np.int64)
            per_core_bwe = []
            for ci in range(c.C):
                adst, asrc = core_edges[ci][d]
                if layer == 0:
                    row = localmap[ci][asrc]
                else:
                    row = (asrc // c.OWN) * c.OWNP + (asrc % c.OWN)
                b = adst // 128
                w = row // c.WIN
                np.add.at(cnt[ci], (b, w), 1)
                per_core_bwe.append((b, w, row - w * c.WIN, adst % 128))
            KT = -(-cnt.max(axis=0) // 128)  # [NB, NW]
            # flat kt offsets in group-major order
            ktoff = np.zeros((c.NB, NW), np.int64)
            acc = 0
            group_meta = []
            for g in range(NG):
                blks = range(g * GB, min((g + 1) * GB, c.NB))
                ops = []
                for w in range(NW):
                    nk = int(sum(KT[b, w] for b in blks))
                    if nk == 0:
                        continue
                    base = acc
                    for b in blks:
                        ktoff[b, w] = acc
                        acc += KT[b, w]
                    ops.append((w, base, nk))
                group_meta.append(ops)
            TOTKT = acc
            scheds.append(dict(NW=NW, KT=KT, ktoff=ktoff, TOTKT=TOTKT,
                               groups=group_meta, cnt=cnt, GB=GB,
                               per_core_bwe=per_core_bwe))
        return scheds

    sched1 = build_sched(0)
    sched2 = build_sched(1)

    def build_slots(sch, per_core_idx):
        """-> (S_host [128, TOTKT*128] bf16, idx [128, TOTKT*8] int16)"""
        b, w, rel, lane = per_core_idx
        KT, ktoff, TOTKT = sch["KT"], sch["ktoff"], sch["TOTKT"]
        # position within (b, w) segment
        order = np.lexsort((np.arange(len(b)), w, b))
        bs, ws, rels, lanes = b[order], w[order], rel[order], lane[order]
        seg = bs * sch["NW"] + ws
        segchange = np.r_[True, seg[1:] != seg[:-1]]
        segstart = np.maximum.accumulate(np.where(segchange, np.arange(len(seg)), 0))
        pos = np.arange(len(seg)) - segstart
        slot = ktoff[bs, ws] * 128 + pos
        nslots = TOTKT * 128
        idx_flat = np.zeros(nslots, np.int64)
        idx_flat[slot] = rels
        S = np.zeros((128, TOTKT * 128), bf16)
        S[slot % 128, (slot // 128) * 128 + lanes] = 1.0
        assert rels.max(initial=0) < 32768
        return S, _rep16(idx_flat.astype(np.int16), nslots)

    # ---- masked-node loss prep
    owner = mask_nodes // c.OWN
    mrows = [mask_nodes[owner == ci] - ci * c.OWN for ci in range(c.C)]
    MP = max(2048, -(-max(len(r) for r in mrows) // 2048) * 2048)

    # ---- shared (replicated) weight inputs
    w1all = np.concatenate([W["on_td_W1"], W["tgt_td_W1"],
                            W["on_bu_W1"], W["tgt_bu_W1"]], axis=1).astype(bf16)
    w2_td = np.concatenate([W["on_td_W2"], W["tgt_td_W2"]], axis=1).astype(bf16)
    w2_bu = np.concatenate([W["on_bu_W2"], W["tgt_bu_W2"]], axis=1).astype(bf16)
    ton = np.concatenate([token @ W["on_td_W1"], token @ W["on_bu_W1"]])
    tonbc = _bcast(ton).astype(bf16)
    b1bc_td = _bcast(np.concatenate([W["on_td_b1"], W["tgt_td_b1"]]))
    b1bc_bu = _bcast(np.concatenate([W["on_bu_b1"], W["tgt_bu_b1"]]))
    b2bc_td = _bcast(np.concatenate([W["on_td_b2"], W["tgt_td_b2"]]))
    b2bc_bu = _bcast(np.concatenate([W["on_bu_b2"], W["tgt_bu_b2"]]))
    ones = np.ones((128, 1), np.float32)
    gmask = np.zeros((128, 1), np.float32)
    gmask[:c.G, 0] = 1.0

    # ---- per-core inputs
    in_maps = []
    for ci in range(c.C):
        lo = ci * c.OWN
        um, mk = needed_um[ci], needed_mk[ci]
        xT = np.zeros((512, RT), bf16)
        xT[:, :len(um)] = xbf[um].T
        xT[:, RU_PAD:RU_PAD + len(mk)] = xbf[mk].T

        def rowarr(dv):
            a = np.ones(RT, np.float32)
            a[:len(um)] = dv[um]
            a[RU_PAD:RU_PAD + len(mk)] = dv[mk]
            return np.ascontiguousarray(a.reshape(-1, 128).T)  # [128, RT//128]

        def dstarr(dv):
            a = np.ones(c.OWNP, np.float32)
            a[:c.OWN] = dv[lo:lo + c.OWN]
            return np.ascontiguousarray(a.reshape(-1, 128).T)  # [128, NB]

        m = dict(xT=xT,
                 dloc_td=rowarr(dinv_td), dloc_bu=rowarr(dinv_bu),
                 ddst_td=dstarr(dinv_td), ddst_bu=dstarr(dinv_bu))
        for d, nm in ((0, "td"), (1, "bu")):
            S, idx = build_slots(sched1[d], sched1[d]["per_core_bwe"][ci])
            m[f"s_{nm}1"], m[f"i_{nm}1"] = S, idx
            S, idx = build_slots(sched2[d], sched2[d]["per_core_bwe"][ci])
            m[f"s_{nm}2"], m[f"i_{nm}2"] = S, idx
        rows = mrows[ci]
        mi = np.zeros(MP, np.int64); mi[:len(rows)] = rows
        mw = np.zeros(MP, np.float32); mw[:len(rows)] = 1.0
        m["midx"] = _rep16(mi.astype(np.int16), MP)
        m["mw"] = np.ascontiguousarray(mw.reshape(-1, 128).T)  # [128, MP//128]
        pp = np.zeros((128, c.NB * 128), np.float32)
        nid = np.arange(c.OWN)
        pp[nid % 128, (nid // 128) * 128 + batch[lo:lo + c.OWN]] = 1.0
        m["ppool"] = pp
        m.update(w1all=w1all, w2_td=w2_td, w2_bu=w2_bu, tonbc=tonbc,
                 b1bc_td=b1bc_td, b1bc_bu=b1bc_bu, b2bc_td=b2bc_td,
                 b2bc_bu=b2bc_bu, ones=ones, gmask=gmask)
        in_maps.append(m)

    meta = dict(RT=RT, RU_PAD=RU_PAD, RK_PAD=RK_PAD, NW1=NW1, MP=MP,
                sched1=sched1, sched2=sched2)
    return meta, in_maps


# ---------------------------------------------------------------- program

def build_program(cfg, meta):
    import concourse.bass as bass
    import concourse.bacc as bacc
    import concourse.mybir as mybir
    import concourse.tile as tile
    from concourse.masks import make_identity

    c = cfg
    RT, RU_PAD, RK_PAD = meta["RT"], meta["RU_PAD"], meta["RK_PAD"]
    MP = meta["MP"]
    f32, bf, i16 = mybir.dt.float32, mybir.dt.bfloat16, mybir.dt.int16
    MUL, ADD = mybir.AluOpType.mult, mybir.AluOpType.add
    SUB = mybir.AluOpType.subtract

    nc = bacc.Bacc("TRN2", target_bir_lowering=False, debug=False,
                   num_devices=c.C)

    def din(name, shape, dt):
        return nc.dram_tensor(name, shape, dt, kind="ExternalInput")

    xT = din("xT", [512, RT], bf)
    dloc = [din("dloc_td", [128, RT // 128], f32), din("dloc_bu", [128, RT // 128], f32)]
    ddst = [din("ddst_td", [128, c.NB], f32), din("ddst_bu", [128, c.NB], f32)]
    s1 = [din("s_td1", [128, meta["sched1"][0]["TOTKT"] * 128], bf),
          din("s_bu1", [128, meta["sched1"][1]["TOTKT"] * 128], bf)]
    i1 = [din("i_td1", [128, meta["sched1"][0]["TOTKT"] * 8], i16),
          din("i_bu1", [128, meta["sched1"][1]["TOTKT"] * 8], i16)]
    s2 = [din("s_td2", [128, meta["sched2"][0]["TOTKT"] * 128], bf),
          din("s_bu2", [128, meta["sched2"][1]["TOTKT"] * 128], bf)]
    i2 = [din("i_td2", [128, meta["sched2"][0]["TOTKT"] * 8], i16),
          din("i_bu2", [128, meta["sched2"][1]["TOTKT"] * 8], i16)]
    midx = din("midx", [128, MP // 16], i16)
    mw_t = din("mw", [128, MP // 128], f32)
    ppool_t = din("ppool", [128, c.NB * 128], f32)
    w1all = din("w1all", [512, 512], bf)
    w2 = [din("w2_td", [128, 128], bf), din("w2_bu", [128, 128], bf)]
    tonbc = din("tonbc", [128, 256], bf)
    b1bc = [din("b1bc_td", [128, 256], f32), din("b1bc_bu", [128, 256], f32)]
    b2bc = [din("b2bc_td", [128, 128], f32), din("b2bc_bu", [128, 128], f32)]
    ones_t = din("ones", [128, 1], f32)
    gmask_t = din("gmask", [128, 1], f32)
    loss_t = nc.dram_tensor("loss", [1, 1], f32, kind="ExternalOutput")

    z_t = nc.dram_tensor("zarr", [RT, 512], bf, kind="Internal")

    with tile.TileContext(nc) as tc:
        with (
            tc.tile_pool(name="const", bufs=1) as cpool,
            tc.tile_pool(name="dram", bufs=1, space="DRAM") as dpool,
        ):
            z2own = [dpool.tile([c.OWNP, 128], bf, tag=f"z2own{d}", name=f"z2own{d}") for d in range(2)]
            z2full = [dpool.tile([c.NPAD, 128], bf, addr_space="Shared", tag=f"z2full{d}", name=f"z2full{d}")
                      for d in range(2)]
            hown = [dpool.tile([c.OWNP, 128], f32, tag=f"hown{d}", name=f"hown{d}") for d in range(2)]
            ar_in = dpool.tile([128, 260], f32, tag="arin", name="arin")
            ar_out = dpool.tile([128, 260], f32, addr_space="Shared", tag="arout", name="arout")

            ident = cpool.tile([128, 128], bf)
            make_identity(nc, ident[:])
            w1sb = cpool.tile([128, 4 * 512], bf)
            for k in range(4):
                nc.sync.dma_start(out=w1sb[:, k * 512:(k + 1) * 512],
                                  in_=w1all[k * 128:(k + 1) * 128, :])
            w2sb = [cpool.tile([128, 128], bf, tag=f"w2_{d}", name=f"w2_{d}") for d in range(2)]
            tonsb = cpool.tile([128, 256], bf)
            b1sb = [cpool.tile([128, 256], f32, tag=f"b1_{d}", name=f"b1_{d}") for d in range(2)]
            b2sb = [cpool.tile([128, 128], f32, tag=f"b2_{d}", name=f"b2_{d}") for d in range(2)]
            dlsb = [cpool.tile([128, RT // 128], f32, tag=f"dl_{d}", name=f"dl_{d}") for d in range(2)]
            ddsb = [cpool.tile([128, c.NB], f32, tag=f"dd_{d}", name=f"dd_{d}") for d in range(2)]
            onesb = cpool.tile([128, 1], f32)
            gmsb = cpool.tile([128, 1], f32)
            nc.sync.dma_start(out=tonsb[:], in_=tonbc[:, :])
            nc.sync.dma_start(out=onesb[:], in_=ones_t[:, :])
            nc.sync.dma_start(out=gmsb[:], in_=gmask_t[:, :])
            for d in range(2):
                nc.sync.dma_start(out=w2sb[d][:], in_=w2[d][:, :])
                nc.sync.dma_start(out=b1sb[d][:], in_=b1bc[d][:, :])
                nc.sync.dma_start(out=b2sb[d][:], in_=b2bc[d][:, :])
                nc.sync.dma_start(out=dlsb[d][:], in_=dloc[d][:, :])
                nc.sync.dma_start(out=ddsb[d][:], in_=ddst[d][:, :])

            # ================= P1: z = scaled([x1|x] @ W1-fused) ==========
            with (
                tc.tile_pool(name="xk", bufs=2) as xkp,
                tc.tile_pool(name="zsb", bufs=3) as zsp,
                tc.tile_pool(name="pz", bufs=2, space="PSUM") as pzp,
            ):
                for sec, (r0, rlen) in enumerate(((0, RU_PAD), (RU_PAD, RK_PAD))):
                    for off in range(0, rlen, c.NF):
                        nf = min(c.NF, rlen - off)
                        xk = xkp.tile([128, 4 * c.NF], bf, tag="xk", name="xk")
                        for k in range(4):
                            nc.sync.dma_start(
                                out=xk[:, k * c.NF:k * c.NF + nf],
                                in_=xT[k * 128:(k + 1) * 128, r0 + off:r0 + off + nf])
                        for j in range(nf // 128):
                            row = r0 + off + j * 128
                            rb = row // 128
                            if sec == 0:
                                ps = pzp.tile([128, 512], f32, tag="pz", name="pz")
                                for k in range(4):
                                    nc.tensor.matmul(
                                        out=ps[:],
                                        lhsT=xk[:, k * c.NF + j * 128:k * c.NF + (j + 1) * 128],
                                        rhs=w1sb[:, k * 512:(k + 1) * 512],
                                        start=(k == 0), stop=(k == 3))
                                zs = zsp.tile([128, 512], bf, tag="zs", name="zs")
                                nc.scalar.activation(
                                    out=zs[:, 0:256], in_=ps[:, 0:256],
                                    func=mybir.ActivationFunctionType.Copy,
                                    scale=dlsb[0][:, rb:rb + 1])
                                nc.scalar.activation(
                                    out=zs[:, 256:512], in_=ps[:, 256:512],
                                    func=mybir.ActivationFunctionType.Copy,
                                    scale=dlsb[1][:, rb:rb + 1])
                            else:
                                ps = pzp.tile([128, 512], f32, tag="pz", name="pz")
                                for k in range(4):
                                    nc.tensor.matmul(
                                        out=ps[:, 0:128],
                                        lhsT=xk[:, k * c.NF + j * 128:k * c.NF + (j + 1) * 128],
                                        rhs=w1sb[:, k * 512 + 128:k * 512 + 256],
                                        start=(k == 0), stop=(k == 3))
                                for k in range(4):
                                    nc.tensor.matmul(
                                        out=ps[:, 128:256],
                                        lhsT=xk[:, k * c.NF + j * 128:k * c.NF + (j + 1) * 128],
                                        rhs=w1sb[:, k * 512 + 384:k * 512 + 512],
                                        start=(k == 0), stop=(k == 3))
                                zs = zsp.tile([128, 512], bf, tag="zs", name="zs")
                                nc.vector.tensor_scalar(
                                    out=zs[:, 0:128], in0=tonsb[:, 0:128],
                                    scalar1=dlsb[0][:, rb:rb + 1], scalar2=None, op0=MUL)
                                nc.scalar.activation(
                                    out=zs[:, 128:256], in_=ps[:, 0:128],
                                    func=mybir.ActivationFunctionType.Copy,
                                    scale=dlsb[0][:, rb:rb + 1])
                                nc.vector.tensor_scalar(
                                    out=zs[:, 256:384], in0=tonsb[:, 128:256],
                                    scalar1=dlsb[1][:, rb:rb + 1], scalar2=None, op0=MUL)
                                nc.scalar.activation(
                                    out=zs[:, 384:512], in_=ps[:, 128:256],
                                    func=mybir.ActivationFunctionType.Copy,
                                    scale=dlsb[1][:, rb:rb + 1])
                            nc.sync.dma_start(out=z_t[row:row + 128, :], in_=zs[:])

            # ================= helper: one aggregation layer ==============
            def agg_layer(layer, d, sch, s_in, i_in, src_t, src_cols, elem,
                          estep, poolps):
                NW, KT, ktoff = sch["NW"], sch["KT"], sch["ktoff"]
                wlen = lambda w: min(c.WIN, (RT if layer == 0 else c.NPAD) - w * c.WIN)
                with (
                    tc.tile_pool(name=f"g{layer}{d}", bufs=2) as gp,
                    tc.tile_pool(name=f"sI{layer}{d}", bufs=2) as sp,
                    tc.tile_pool(name=f"ix{layer}{d}", bufs=2) as ip,
                    tc.tile_pool(name=f"fin{layer}{d}", bufs=3) as fp,
                    tc.tile_pool(name=f"pp{layer}{d}", bufs=2) as ppp,
                    tc.tile_pool(name=f"agg{layer}{d}", bufs=2, space="PSUM") as ap,
                    tc.tile_pool(name=f"tr{layer}{d}", bufs=2, space="PSUM") as trp,
                ):
                    GB = sch["GB"]
                    for g, ops in enumerate(sch["groups"]):
                        blks = range(g * GB, min((g + 1) * GB, c.NB))
                        if not ops:
                            continue
                        gkt0 = ops[0][1]
                        gnkt = sum(nk for _, _, nk in ops)
                        st = sp.tile([128, gnkt * 128], bf, tag="s", name="s")
                        nc.sync.dma_start(
                            out=st[:], in_=s_in[:, gkt0 * 128:(gkt0 + gnkt) * 128])
                        it = ip.tile([128, gnkt * 8], i16, tag="i", name="i")
                        nc.sync.dma_start(
                            out=it[:], in_=i_in[:, gkt0 * 8:(gkt0 + gnkt) * 8])
                        gt = gp.tile([128, gnkt * elem], bf, tag="g", name="g")
                        optiles = {}
                        for w, base, nk in ops:
                            o = base - gkt0
                            nc.gpsimd.dma_gather(
                                gt[:, o * elem:(o + nk) * elem].rearrange(
                                    "p (k e) -> p k e", k=nk, e=elem),
                                src_t[w * c.WIN:w * c.WIN + wlen(w),
                                      src_cols[0]:src_cols[1]],
                                it[:, o * 8:(o + nk) * 8], nk * 128, nk * 128, elem,
                                elem_step=estep, single_packet=False)
                            optiles[w] = (gt, gkt0)
                        if layer == 0:
                            ptile = None
                        else:
                            ptile = ppp.tile([128, len(blks) * 128], f32, tag="pp", name="pp")
                            b0 = g * GB
                            nc.sync.dma_start(
                                out=ptile[:],
                                in_=ppool_t[:, b0 * 128:(b0 + len(blks)) * 128])
                        for b in blks:
                            nkb = int(KT[b].sum())
                            if nkb == 0:
                                continue
                            fw = 256 if layer == 0 else 128
                            ps = ap.tile([128, fw], f32, tag="a", name="a")
                            emitted = 0
                            for w in range(NW):
                                if KT[b, w] == 0:
                                    continue
                                gt, base = optiles[w]
                                for t in range(KT[b, w]):
                                    kt = ktoff[b, w] + t
                                    nc.tensor.matmul(
                                        out=ps[:],
                                        lhsT=st[:, (kt - gkt0) * 128:(kt - gkt0 + 1) * 128],
                                        rhs=gt[:, (kt - base) * elem:(kt - base + 1) * elem],
                                        start=(emitted == 0),
                                        stop=(emitted == nkb - 1))
                                    emitted += 1
                            # finalize: (ps * dinv_dst) + bias
                            bias = b1sb[d] if layer == 0 else b2sb[d]
                            nc.vector.scalar_tensor_tensor(
                                out=ps[:], in0=ps[:], scalar=ddsb[d][:, b:b + 1],
                                in1=bias[:, 0:fw], op0=MUL, op1=ADD)
                            if layer == 0:
                                h1 = fp.tile([128, 256], bf, tag="h1", name="h1")
                                nc.scalar.activation(
                                    out=h1[:], in_=ps[:],
                                    func=mybir.ActivationFunctionType.Relu)
                                trt = trp.tile([128, 256], bf, tag="t", name="t")
                                nc.tensor.transpose(
                                    out=trt[:, 0:128], in_=h1[:, 0:128], identity=ident[:])
                                nc.tensor.transpose(
                                    out=trt[:, 128:256], in_=h1[:, 128:256], identity=ident[:])
                                h1T = fp.tile([128, 256], bf, tag="h1T", name="h1T")
                                nc.vector.tensor_copy(out=h1T[:], in_=trt[:])
                                z2ps = trp.tile([128, 128], f32, tag="z2", name="z2")
                                nc.tensor.matmul(out=z2ps[:, 0:64],
                                                 lhsT=h1T[:, 0:128],
                                                 rhs=w2sb[d][:, 0:64],
                                                 start=True, stop=True)
                                nc.tensor.matmul(out=z2ps[:, 64:128],
                                                 lhsT=h1T[:, 128:256],
                                                 rhs=w2sb[d][:, 64:128],
                                                 start=True, stop=True)
                                z2sb = fp.tile([128, 128], bf, tag="z2sb", name="z2sb")
                                nc.scalar.activation(
                                    out=z2sb[:], in_=z2ps[:],
                                    func=mybir.ActivationFunctionType.Copy,
                                    scale=ddsb[d][:, b:b + 1])
                                nc.sync.dma_start(
                                    out=z2own[d][b * 128:(b + 1) * 128, :], in_=z2sb[:])
                            else:
                                hsb = fp.tile([128, 128], f32, tag="hsb", name="hsb")
                                nc.scalar.copy(out=hsb[:], in_=ps[:])
                                nc.tensor.matmul(
                                    out=poolps[:], lhsT=ptile[:, (b - g * GB) * 128:(b - g * GB + 1) * 128],
                                    rhs=hsb[:], start=(b == 0), stop=(b == c.NB - 1),
                                    skip_group_check=True)
                                nc.sync.dma_start(
                                    out=hown[d][b * 128:(b + 1) * 128, :], in_=hsb[:])

            # ================= L1 (both dirs) =============================
            for d in range(2):
                agg_layer(0, d, meta["sched1"][d], s1[d], i1[d],
                          z_t, (256 * d, 256 * d + 256), 256, 512, None)

            # ================= AllGather z2 ===============================
            for d in range(2):
                nc.gpsimd.collective_compute(
                    "AllGather", mybir.AluOpType.bypass,
                    replica_groups=[list(range(c.C))],
                    ins=[z2own[d].opt()], outs=[z2full[d].opt()])

            # ================= L2 (both dirs) =============================
            with tc.tile_pool(name="plps", bufs=2, space="PSUM") as plp:
                poolps = [plp.tile([128, 128], f32, tag=f"pl{d}", name=f"pl{d}") for d in range(2)]
                for d in range(2):
                    agg_layer(1, d, meta["sched2"][d], s2[d], i2[d],
                              z2full[d], (0, 128), 128, None, poolps[d])

                # ============= masked-node cosine partial ================
                def cos_terms(sp_, a1, a2, b1_, b2_, tag):
                    """-> (dot, n1, n2) [128,1] f32 tiles"""
                    outs = []
                    for (u, v) in ((a1, a2), (a1, a1), (a2, a2)):
                        acc1 = sp_.tile([128, 1], f32, tag=f"{tag}ac1", name=f"{tag}ac1")
                        acc2 = sp_.tile([128, 1], f32, tag=f"{tag}ac2", name=f"{tag}ac2")
                        scr = sp_.tile([128, 64], f32, tag=f"{tag}scr", name=f"{tag}scr")
                        nc.vector.scalar_tensor_tensor(
                            out=scr[:], in0=u[0], scalar=1.0, in1=v[0],
                            op0=MUL, op1=MUL, accum_out=acc1[:])
                        nc.vector.scalar_tensor_tensor(
                            out=scr[:], in0=u[1], scalar=1.0, in1=v[1],
                            op0=MUL, op1=MUL, accum_out=acc2[:])
                        s = sp_.tile([128, 1], f32, tag=f"{tag}s", name=f"{tag}s")
                        nc.vector.tensor_tensor(out=s[:], in0=acc1[:], in1=acc2[:], op=ADD)
                        outs.append(s)
                    return outs

                def rcp_guard(sp_, n, tag):
                    r = sp_.tile([128, 1], f32, tag=f"{tag}r", name=f"{tag}r")
                    nc.scalar.sqrt(out=r[:], in_=n[:])
                    nc.vector.tensor_scalar_max(out=r[:], in0=r[:], scalar1=1e-12)
                    nc.vector.reciprocal(out=r[:], in_=r[:])
                    return r

                with (
                    tc.tile_pool(name="msk", bufs=2) as mp_,
                    tc.tile_pool(name="msc", bufs=4) as sc_,
                    tc.tile_pool(name="scps", bufs=2, space="PSUM") as scp,
                ):
                    macc = cpool.tile([128, 1], f32)
                    nc.vector.memset(macc[:], 0.0)
                    MOPS = MP // 2048
                    for o in range(MOPS):
                        it = mp_.tile([128, 128], i16, tag="mi", name="mi")
                        nc.sync.dma_start(out=it[:], in_=midx[:, o * 128:(o + 1) * 128])
                        wt = mp_.tile([128, 16], f32, tag="mwt", name="mwt")
                        nc.sync.dma_start(out=wt[:], in_=mw_t[:, o * 16:(o + 1) * 16])
                        gts = []
                        for d in range(2):
                            gt = mp_.tile([128, 16 * 128], f32, tag=f"mg{d}", name=f"mg{d}")
                            nc.gpsimd.dma_gather(
                                gt[:].rearrange("p (k e) -> p k e", k=16, e=128),
                                hown[d][:, :], it[:], 2048, 2048, 128,
                                elem_step=None, single_packet=False)
                            gts.append(gt)
                        for k in range(16):
                            atd = gts[0][:, k * 128:(k + 1) * 128]
                            abu = gts[1][:, k * 128:(k + 1) * 128]
                            dot, n1, n2 = cos_terms(
                                sc_, (atd[:, 0:64], abu[:, 0:64]),
                                (atd[:, 64:128], abu[:, 64:128]),
                                None, None, "m")
                            r1 = rcp_guard(sc_, n1, "m1")
                            r2 = rcp_guard(sc_, n2, "m2")
                            cosv = sc_.tile([128, 1], f32, tag="mcos", name="mcos")
                            nc.vector.tensor_tensor(out=cosv[:], in0=dot[:], in1=r1[:], op=MUL)
                            nc.vector.tensor_tensor(out=cosv[:], in0=cosv[:], in1=r2[:], op=MUL)
                            u = sc_.tile([128, 1], f32, tag="mu", name="mu")
                            nc.vector.tensor_tensor(out=u[:], in0=cosv[:],
                                                    in1=wt[:, k:k + 1], op=MUL)
                            term = sc_.tile([128, 1], f32, tag="mt", name="mt")
                            nc.vector.tensor_tensor(out=term[:], in0=wt[:, k:k + 1],
                                                    in1=u[:], op=SUB)
                            nc.vector.tensor_tensor(out=macc[:], in0=macc[:],
                                                    in1=term[:], op=ADD)
                    msps = scp.tile([1, 1], f32, tag="ms", name="ms")
                    nc.tensor.matmul(out=msps[:], lhsT=macc[:], rhs=onesb[:],
                                     start=True, stop=True)

                    # ============= pool partials -> AllReduce ============
                    arsb = cpool.tile([128, 260], f32)
                    nc.vector.memset(arsb[:], 0.0)
                    nc.vector.tensor_copy(out=arsb[:, 0:128], in_=poolps[0][:])
                    nc.vector.tensor_copy(out=arsb[:, 128:256], in_=poolps[1][:])
                    nc.vector.tensor_copy(out=arsb[0:1, 256:257], in_=msps[:])
                    nc.sync.dma_start(out=ar_in[:, :], in_=arsb[:])
                    nc.gpsimd.collective_compute(
                        "AllReduce", mybir.AluOpType.add,
                        replica_groups=[list(range(c.C))],
                        ins=[ar_in.opt()], outs=[ar_out.opt()])
                    ar2 = cpool.tile([128, 260], f32)
                    nc.sync.dma_start(out=ar2[:], in_=ar_out[:, :])

                    # ============= pooled cosine + final loss ============
                    dot, n1, n2 = cos_terms(
                        sc_, (ar2[:, 0:64], ar2[:, 128:192]),
                        (ar2[:, 64:128], ar2[:, 192:256]), None, None, "g")
                    r1 = rcp_guard(sc_, n1, "g1")
                    r2 = rcp_guard(sc_, n2, "g2")
                    cosg = sc_.tile([128, 1], f32, tag="gcos", name="gcos")
                    nc.vector.tensor_tensor(out=cosg[:], in0=dot[:], in1=r1[:], op=MUL)
                    nc.vector.tensor_tensor(out=cosg[:], in0=cosg[:], in1=r2[:], op=MUL)
                    gterm = sc_.tile([128, 1], f32, tag="gt", name="gt")
                    nc.vector.tensor_scalar(out=gterm[:], in0=cosg[:],
                                            scalar1=-1.0, scalar2=1.0,
                                            op0=MUL, op1=ADD)
                    nc.vector.tensor_tensor(out=gterm[:], in0=gterm[:],
                                            in1=gmsb[:], op=MUL)
                    gsps = scp.tile([1, 1], f32, tag="gs", name="gs")
                    nc.tensor.matmul(out=gsps[:], lhsT=gterm[:], rhs=onesb[:],
                                     start=True, stop=True)
                    l1t = sc_.tile([1, 1], f32, tag="l1", name="l1")
                    nc.scalar.activation(out=l1t[:], in_=gsps[:],
                                         func=mybir.ActivationFunctionType.Copy,
                                         scale=1.0 / c.G)
                    l2t = sc_.tile([1, 1], f32, tag="l2", name="l2")
                    nc.scalar.activation(out=l2t[:], in_=ar2[0:1, 256:257],
                                         func=mybir.ActivationFunctionType.Copy,
                                         scale=1.0 / c.M)
                    nc.vector.tensor_tensor(out=l1t[:], in0=l1t[:], in1=l2t[:], op=ADD)
                    nc.sync.dma_start(out=loss_t[:, :], in_=l1t[:])

    return nc


# ---------------------------------------------------------------- entry

LAST_RESULT = None


def _install_trace_hook():
    """The agent image's antenv lacks axon_hooks; synthesize it from
    trn_boot's ctypes NTFF hook so trace=True works under axon."""
    import types
    try:
        from antenv import axon_hooks  # noqa: F401
        return
    except ImportError:
        pass
    try:
        import antenv
        import trn_agent_boot.trn_boot as tb
        hook = tb._ntff_profile_via_ctypes("/opt/axon/libaxon_pjrt.so")
        mod = types.ModuleType("antenv.axon_hooks")
        mod.get_axon_ntff_profile_hook = lambda: hook
        mod.set_axon_ntff_profile_hook = lambda h: None
        sys.modules["antenv.axon_hooks"] = mod
        antenv.axon_hooks = mod
    except Exception as e:
        print(f"[kernel] trace hook install failed: {e}", file=sys.stderr)


def kernel(_trace=False, **inputs):
    global LAST_RESULT
    import time
    from concourse import bass_utils
    if _trace:
        _install_trace_hook()
    cfg = FULL
    t0 = time.monotonic()
    meta, in_maps = host_prep(cfg, inputs)
    t1 = time.monotonic()
    nc = build_program(cfg, meta)
    t2 = time.monotonic()
    nc.compile()
    t3 = time.monotonic()
    res = bass_utils.run_bass_kernel_spmd(
        nc, in_maps, core_ids=list(range(cfg.C)),
        trace=_trace, trace_cores=[0] if _trace else None)
    t4 = time.monotonic()
    print(f"[kernel] prep {t1-t0:.1f}s build {t2-t1:.1f}s "
          f"compile {t3-t2:.1f}s run {t4-t3:.1f}s", file=sys.stderr)
    LAST_RESULT = res
    return np.float32(res.results[0]["loss"][0, 0])



# revision 53
# speedup vs baseline: 3.6836x; 3.6836x over previous
"""Trainium2 Bass kernel for the rumor-GCN masked-autoencoder loss.

Strategy (8 NeuronCores, SPMD single NEFF):
  - Nodes are partitioned into 8 contiguous ranges (25000 each). Each core
    aggregates the in-edges of its own nodes (TD: grouped by dst, BU: grouped
    by src) -- "halo exchange" is done by the host pre-gathering the x-rows
    each core needs, so layer-1 is fully local.
  - Layer-1 linear is one fused [512->512] bf16 matmul over each core's
    needed-node set (4 GCN heads side by side). GCN symmetric norms are
    folded into per-row scales (dinv[src] into z at the P1 copy-out, dinv[dst]
    at aggregation finalize), so the sparse selection matrices are pure 0/1.
  - Edge aggregation: dma_gather pulls source rows into K-tiles of 128 edge
    slots; a one-hot S matrix per K-tile turns scatter-add into PE matmuls
    accumulating in PSUM per 128-dst-node block.
  - Layer 2 never materializes h2 for all nodes. Instead:
      * global_add_pool is rewritten src-side: pool[g] += w_e * z2[src_e]
        with w_e = dinv[dst] grouped by graph(dst), which makes it a dense
        matmul over z2own tiles against a host-built value-S ("poolS") —
        zero runtime gathers for pooling.
      * h2 is aggregated only for OWN masked nodes (the sole per-node
        consumers), feature-major ([feat, mask-col] PSUMs), gathering z2
        halo rows from the AllGathered z2full; scatter matrices are built
        on-device by is_equal(iota, dstcol)*w from tiny per-slot metadata.
        Self-loops gather from the local z2own table (single int16 window)
        to avoid max-over-core window padding.
  - Each direction's z2 AllGather is issued as soon as that direction's L1
    finishes, overlapping the other direction's aggregation.
  - Masked cosine terms reduce per column via ones-matmuls over stacked
    products; pooled sums + the mask partial go through one small
    AllReduce; every core computes the identical final scalar loss.
  (dma_gather costs ~8.4ns/index serialized on GpSimd regardless of payload
   size, so total gather-index count — incl. schedule padding — is the
   primary cost driver; see /root/problem/microbench.py.)
"""

import sys

import numpy as np

sys.path.insert(0, "/opt/trn_rl_repo")

# ---------------------------------------------------------------- config

class Cfg:
    def __init__(self, N, E, G, M, C=8, WIN=32768, GB=8, GB2=8, NF=2048):
        self.N, self.E, self.G, self.M, self.C = N, E, G, M, C
        self.IN, self.HID, self.OUT = 512, 128, 64
        self.WIN, self.GB, self.GB2, self.NF = WIN, GB, GB2, NF
        assert N % C == 0
        self.OWN = N // C
        self.NB = -(-self.OWN // 128)
        self.OWNP = self.NB * 128
        self.NPAD = C * self.OWNP
        self.NW2 = -(-self.NPAD // WIN)


FULL = Cfg(N=200000, E=400000, G=128, M=100000)

_WNAMES = [p + s for p in ("on_td", "on_bu", "tgt_td", "tgt_bu")
           for s in ("_W1", "_b1", "_W2", "_b2")]


def _rep16(idx_flat, nslots):
    """int16 index list -> [128, nslots//16] layout (16-part wrap, 8x replicated)."""
    blk = np.zeros((16, nslots // 16), dtype=np.int16)
    k = np.arange(len(idx_flat))
    blk[k % 16, k // 16] = idx_flat
    return np.tile(blk, (8, 1))


def _bcast(vec, parts=128):
    return np.broadcast_to(np.asarray(vec)[None, :], (parts, len(vec))).copy()


# ---------------------------------------------------------------- host prep

def host_prep(cfg, inp):
    import ml_dtypes
    bf16 = ml_dtypes.bfloat16
    c = cfg
    x = np.asarray(inp["x"], np.float32)
    token = np.asarray(inp["enc_mask_token"], np.float32).reshape(-1)
    ei = np.asarray(inp["edge_index"])
    src, dst = ei[0].astype(np.int64), ei[1].astype(np.int64)
    batch = np.asarray(inp["batch"]).astype(np.int64)
    mask_nodes = np.asarray(inp["mask_nodes"]).astype(np.int64)
    W = {k: np.asarray(inp[k], np.float32) for k in _WNAMES}

    dinv_td = (1.0 / np.sqrt(np.bincount(dst, minlength=c.N) + 1.0)).astype(np.float32)
    dinv_bu = (1.0 / np.sqrt(np.bincount(src, minlength=c.N) + 1.0)).astype(np.float32)
    is_masked = np.zeros(c.N, bool)
    is_masked[mask_nodes] = True
    xbf = x.astype(bf16)

    # ---- per-core edge lists (agg_dst local block/lane, agg_src global)
    # dir 0 = TD (aggregate src -> dst, dinv_td); dir 1 = BU (dst -> src, dinv_bu)
    core_edges = []   # [core][dir] -> (adst_local, asrc_global)
    for ci in range(c.C):
        lo, hi = ci * c.OWN, (ci + 1) * c.OWN
        per = []
        for d in range(2):
            ad, as_ = (dst, src) if d == 0 else (src, dst)
            sel = (ad >= lo) & (ad < hi)
            adst = ad[sel] - lo
            asrc = as_[sel]
            # self loops
            adst = np.concatenate([adst, np.arange(c.OWN, dtype=np.int64)])
            asrc = np.concatenate([asrc, np.arange(lo, hi, dtype=np.int64)])
            per.append((adst, asrc))
        core_edges.append(per)

    # ---- needed sets / z-row maps
    RU, RK = [], []
    needed_um, needed_mk, localmap = [], [], []
    for ci in range(c.C):
        lo, hi = ci * c.OWN, (ci + 1) * c.OWN
        nodes = np.unique(np.concatenate(
            [np.arange(lo, hi, dtype=np.int64),
             core_edges[ci][0][1], core_edges[ci][1][1]]))
        um = nodes[~is_masked[nodes]]
        mk = nodes[is_masked[nodes]]
        needed_um.append(um); needed_mk.append(mk)
        RU.append(len(um)); RK.append(len(mk))
    RU_PAD = -(-max(RU) // 128) * 128
    RK_PAD = -(-max(RK) // 128) * 128
    RT = RU_PAD + RK_PAD
    NW1 = -(-RT // c.WIN)
    for ci in range(c.C):
        lm = np.full(c.N, -1, np.int64)
        lm[needed_um[ci]] = np.arange(RU[ci])
        lm[needed_mk[ci]] = RU_PAD + np.arange(RK[ci])
        localmap.append(lm)

    # ---- slot schedules: for (dir, layer) build common KT[b][w], flat kt order
    # flat order: group g -> window w -> block b in g -> tiles
    def build_sched(layer):
        GB = c.GB if layer == 0 else c.GB2
        NG = -(-c.NB // GB)
        scheds = []
        for d in range(2):
            NW = NW1 if layer == 0 else c.NW2
            cnt = np.zeros((c.C, c.NB, NW), np.int64)
            per_core_bwe = []
            for ci in range(c.C):
                adst, asrc = core_edges[ci][d]
                if layer == 0:
                    row = localmap[ci][asrc]
                else:
                    row = (asrc // c.OWN) * c.OWNP + (asrc % c.OWN)
                b = adst // 128
                w = row // c.WIN
                np.add.at(cnt[ci], (b, w), 1)
                per_core_bwe.append((b, w, row - w * c.WIN, adst % 128))
            KT = -(-cnt.max(axis=0) // 128)  # [NB, NW]
            # flat kt offsets in group-major order
            ktoff = np.zeros((c.NB, NW), np.int64)
            acc = 0
            group_meta = []
            for g in range(NG):
                blks = range(g * GB, min((g + 1) * GB, c.NB))
                ops = []
                for w in range(NW):
                    nk = int(sum(KT[b, w] for b in blks))
                    if nk == 0:
                        continue
                    base = acc
                    for b in blks:
                        ktoff[b, w] = acc
                        acc += KT[b, w]
                    ops.append((w, base, nk))
                group_meta.append(ops)
            TOTKT = acc
            scheds.append(dict(NW=NW, KT=KT, ktoff=ktoff, TOTKT=TOTKT,
                               groups=group_meta, cnt=cnt, GB=GB,
                               per_core_bwe=per_core_bwe))
        return scheds

    sched1 = build_sched(0)

    # ---- mask-aggregation schedule: only in-edges (incl self) of OWN masked
    # nodes, gathered from z2full; dst = position in the per-core compact
    # masked-node list, grouped in bgroups of 512 columns.
    NW2 = -(-c.NPAD // c.WIN)
    mk_cnt = [int(is_masked[ci * c.OWN:(ci + 1) * c.OWN].sum()) for ci in range(c.C)]
    MBG = -(-max(mk_cnt) // 512)
    mcnt_global = np.bincount(mask_nodes, minlength=c.N)
    msched = []
    for d in range(2):
        ad_g, as_g = (dst, src) if d == 0 else (src, dst)
        dv = dinv_td if d == 0 else dinv_bu
        cnt = np.zeros((c.C, MBG, NW2), np.int64)
        scnt = np.zeros((c.C, MBG), np.int64)
        per_core = []
        for ci in range(c.C):
            lo = ci * c.OWN
            mloc = np.where(is_masked[lo:lo + c.OWN])[0]
            mpos_local = np.full(c.OWN, -1, np.int64)
            mpos_local[mloc] = np.arange(len(mloc))
            sel = (ad_g >= lo) & (ad_g < lo + c.OWN) & is_masked[np.clip(ad_g, 0, c.N - 1)]
            adst = ad_g[sel] - lo
            md = mpos_local[adst]
            row = (as_g[sel] // c.OWN) * c.OWNP + (as_g[sel] % c.OWN)
            wv = dv[lo + adst]
            b = md // 512
            w = row // c.WIN
            np.add.at(cnt[ci], (b, w), 1)
            smd = np.arange(len(mloc))
            np.add.at(scnt[ci], (smd // 512,), 1)
            per_core.append((b, w, row - w * c.WIN, md % 512, wv,
                             smd // 512, mloc, smd % 512, dv[lo + mloc]))
        KT = -(-cnt.max(axis=0) // 128)   # [MBG, NW2]
        SKT = -(-scnt.max(axis=0) // 128)  # [MBG]
        ktoff = np.zeros((MBG, NW2), np.int64)
        acc = 0
        for b in range(MBG):
            for w in range(NW2):
                ktoff[b, w] = acc
                acc += KT[b, w]
        skoff = np.zeros(MBG, np.int64)
        sacc = 0
        for b in range(MBG):
            skoff[b] = sacc
            sacc += SKT[b]
        msched.append(dict(KT=KT, ktoff=ktoff, TOT=acc, SKT=SKT, skoff=skoff,
                           STOT=sacc, per_core=per_core))

    def _slot_arrays(bs_, ws_, rels, cols, wvs, ktoff2d, TOT, nwcols):
        """scatter slot data into flat [TOT*128] arrays via segment packing"""
        order = np.lexsort((np.arange(len(bs_)), ws_, bs_))
        bs, ws = bs_[order], ws_[order]
        rels, cols, wvs = rels[order], cols[order], wvs[order]
        seg = bs * nwcols + ws
        segchange = np.r_[True, seg[1:] != seg[:-1]]
        segstart = np.maximum.accumulate(np.where(segchange, np.arange(len(seg)), 0))
        pos = np.arange(len(seg)) - segstart
        slot = ktoff2d[bs, ws] * 128 + pos
        nslots = TOT * 128
        idx_flat = np.zeros(nslots, np.int64)
        idx_flat[slot] = rels
        dc = np.full(nslots, -1.0, np.float32)
        dc[slot] = cols
        wvf = np.zeros(nslots, np.float32)
        wvf[slot] = wvs
        assert len(rels) == 0 or rels.max() < 32768
        return (_rep16(idx_flat.astype(np.int16), nslots),
                np.ascontiguousarray(dc.reshape(-1, 128).T),
                np.ascontiguousarray(wvf.reshape(-1, 128).T))

    def build_mask_slots(sch, per):
        b, w, rel, col, wv, sb, srow, scol, swv = per
        halo = _slot_arrays(b, w, rel, col, wv, sch["ktoff"], sch["TOT"], NW2)
        selfa = _slot_arrays(sb, np.zeros_like(sb), srow, scol, swv,
                             sch["skoff"][:, None], sch["STOT"], 1)
        return halo + selfa

    # ---- pool S (values) per dir: out-edge + self weights grouped by graph
    def build_poolS(ci, d):
        import ml_dtypes
        lo = ci * c.OWN
        ad, as_ = (dst, src) if d == 0 else (src, dst)
        dv = dinv_td if d == 0 else dinv_bu
        sel = (as_ >= lo) & (as_ < lo + c.OWN)
        j = as_[sel] - lo
        g = batch[ad[sel]]
        v = dv[ad[sel]]
        pp = np.zeros((128, c.NB * 128), np.float32)
        np.add.at(pp, (j % 128, (j // 128) * 128 + g), v)
        jj = np.arange(c.OWN)
        np.add.at(pp, (jj % 128, (jj // 128) * 128 + batch[lo + jj]),
                  dv[lo + jj])
        return pp.astype(ml_dtypes.bfloat16)

    def build_slots(sch, per_core_idx):
        """-> (S_host [128, TOTKT*128] bf16, idx [128, TOTKT*8] int16)"""
        b, w, rel, lane = per_core_idx
        KT, ktoff, TOTKT = sch["KT"], sch["ktoff"], sch["TOTKT"]
        # position within (b, w) segment
        order = np.lexsort((np.arange(len(b)), w, b))
        bs, ws, rels, lanes = b[order], w[order], rel[order], lane[order]
        seg = bs * sch["NW"] + ws
        segchange = np.r_[True, seg[1:] != seg[:-1]]
        segstart = np.maximum.accumulate(np.where(segchange, np.arange(len(seg)), 0))
        pos = np.arange(len(seg)) - segstart
        slot = ktoff[bs, ws] * 128 + pos
        nslots = TOTKT * 128
        idx_flat = np.zeros(nslots, np.int64)
        idx_flat[slot] = rels
        S = np.zeros((128, TOTKT * 128), bf16)
        S[slot % 128, (slot // 128) * 128 + lanes] = 1.0
        assert rels.max(initial=0) < 32768
        return S, _rep16(idx_flat.astype(np.int16), nslots)

    # ---- shared (replicated) weight inputs
    w1all = np.concatenate([W["on_td_W1"], W["tgt_td_W1"],
                            W["on_bu_W1"], W["tgt_bu_W1"]], axis=1).astype(bf16)
    w2_td = np.concatenate([W["on_td_W2"], W["tgt_td_W2"]], axis=1).astype(bf16)
    w2_bu = np.concatenate([W["on_bu_W2"], W["tgt_bu_W2"]], axis=1).astype(bf16)
    ton = np.concatenate([token @ W["on_td_W1"], token @ W["on_bu_W1"]])
    tonbc = _bcast(ton).astype(bf16)
    b1bc_td = _bcast(np.concatenate([W["on_td_b1"], W["tgt_td_b1"]]))
    b1bc_bu = _bcast(np.concatenate([W["on_bu_b1"], W["tgt_bu_b1"]]))
    b2col = np.stack(
        [W["on_td_b2"], W["tgt_td_b2"], W["on_bu_b2"], W["tgt_bu_b2"]],
        axis=1).astype(np.float32)                         # [64, 4]
    ones = np.ones((128, 1), np.float32)
    iota512 = np.broadcast_to(
        np.arange(512, dtype=np.float32)[None, :], (128, 512)).copy()
    gcount = np.bincount(batch, minlength=c.G).astype(np.float32)
    cntbc = np.broadcast_to(gcount[None, :128], (128, 128)).copy()

    # ---- per-core inputs
    in_maps = []
    for ci in range(c.C):
        lo = ci * c.OWN
        um, mk = needed_um[ci], needed_mk[ci]
        xT = np.zeros((512, RT), bf16)
        xT[:, :len(um)] = xbf[um].T
        xT[:, RU_PAD:RU_PAD + len(mk)] = xbf[mk].T

        def rowarr(dv):
            a = np.ones(RT, np.float32)
            a[:len(um)] = dv[um]
            a[RU_PAD:RU_PAD + len(mk)] = dv[mk]
            return np.ascontiguousarray(a.reshape(-1, 128).T)  # [128, RT//128]

        def dstarr(dv):
            a = np.ones(c.OWNP, np.float32)
            a[:c.OWN] = dv[lo:lo + c.OWN]
            return np.ascontiguousarray(a.reshape(-1, 128).T)  # [128, NB]

        m = dict(xT=xT,
                 dloc_td=rowarr(dinv_td), dloc_bu=rowarr(dinv_bu),
                 ddst_td=dstarr(dinv_td), ddst_bu=dstarr(dinv_bu))
        for d, nm in ((0, "td"), (1, "bu")):
            S, idx = build_slots(sched1[d], sched1[d]["per_core_bwe"][ci])
            m[f"s_{nm}1"], m[f"i_{nm}1"] = S, idx
            mi, mdc, mwv, si, sdc, swv = build_mask_slots(
                msched[d], msched[d]["per_core"][ci])
            m[f"mi_{nm}"], m[f"mdc_{nm}"], m[f"mwv_{nm}"] = mi, mdc, mwv
            m[f"si_{nm}"], m[f"sdc_{nm}"], m[f"swv_{nm}"] = si, sdc, swv
            m[f"pools_{nm}"] = build_poolS(ci, d)
        # per-masked-node multiplicity (for the sce mean over 100k entries)
        mloc = np.where(is_masked[lo:lo + c.OWN])[0]
        cval = np.zeros(MBG * 512, np.float32)
        cval[:len(mloc)] = mcnt_global[lo + mloc]
        m["mcv"] = cval.reshape(1, -1)
        m.update(w1all=w1all, w2_td=w2_td, w2_bu=w2_bu, tonbc=tonbc,
                 b1bc_td=b1bc_td, b1bc_bu=b1bc_bu, b2col=b2col,
                 ones=ones, iota512=iota512, cntbc=cntbc)
        in_maps.append(m)

    meta = dict(RT=RT, RU_PAD=RU_PAD, RK_PAD=RK_PAD, NW1=NW1,
                sched1=sched1, msched=msched, MBG=MBG, NW2=NW2)
    return meta, in_maps


# ---------------------------------------------------------------- program

def build_program(cfg, meta):
    import concourse.bass as bass
    import concourse.bacc as bacc
    import concourse.mybir as mybir
    import concourse.tile as tile
    from concourse.masks import make_identity

    c = cfg
    RT, RU_PAD, RK_PAD = meta["RT"], meta["RU_PAD"], meta["RK_PAD"]
    f32, bf, i16 = mybir.dt.float32, mybir.dt.bfloat16, mybir.dt.int16
    MUL, ADD = mybir.AluOpType.mult, mybir.AluOpType.add
    SUB = mybir.AluOpType.subtract
    ISEQ = mybir.AluOpType.is_equal

    nc = bacc.Bacc("TRN2", target_bir_lowering=False, debug=False,
                   num_devices=c.C)

    def din(name, shape, dt):
        return nc.dram_tensor(name, shape, dt, kind="ExternalInput")

    xT = din("xT", [512, RT], bf)
    dloc = [din("dloc_td", [128, RT // 128], f32), din("dloc_bu", [128, RT // 128], f32)]
    ddst = [din("ddst_td", [128, c.NB], f32), din("ddst_bu", [128, c.NB], f32)]
    s1 = [din("s_td1", [128, meta["sched1"][0]["TOTKT"] * 128], bf),
          din("s_bu1", [128, meta["sched1"][1]["TOTKT"] * 128], bf)]
    i1 = [din("i_td1", [128, meta["sched1"][0]["TOTKT"] * 8], i16),
          din("i_bu1", [128, meta["sched1"][1]["TOTKT"] * 8], i16)]
    MBG, NW2 = meta["MBG"], meta["NW2"]
    mi_t = [din("mi_td", [128, meta["msched"][0]["TOT"] * 8], i16),
            din("mi_bu", [128, meta["msched"][1]["TOT"] * 8], i16)]
    mdc_t = [din("mdc_td", [128, meta["msched"][0]["TOT"]], f32),
             din("mdc_bu", [128, meta["msched"][1]["TOT"]], f32)]
    mwv_t = [din("mwv_td", [128, meta["msched"][0]["TOT"]], f32),
             din("mwv_bu", [128, meta["msched"][1]["TOT"]], f32)]
    si_t = [din("si_td", [128, meta["msched"][0]["STOT"] * 8], i16),
            din("si_bu", [128, meta["msched"][1]["STOT"] * 8], i16)]
    sdc_t = [din("sdc_td", [128, meta["msched"][0]["STOT"]], f32),
             din("sdc_bu", [128, meta["msched"][1]["STOT"]], f32)]
    swv_t = [din("swv_td", [128, meta["msched"][0]["STOT"]], f32),
             din("swv_bu", [128, meta["msched"][1]["STOT"]], f32)]
    pools_t = [din("pools_td", [128, c.NB * 128], bf),
               din("pools_bu", [128, c.NB * 128], bf)]
    mcv_t = din("mcv", [1, MBG * 512], f32)
    w1all = din("w1all", [512, 512], bf)
    w2 = [din("w2_td", [128, 128], bf), din("w2_bu", [128, 128], bf)]
    tonbc = din("tonbc", [128, 256], bf)
    b1bc = [din("b1bc_td", [128, 256], f32), din("b1bc_bu", [128, 256], f32)]
    b2col_t = din("b2col", [64, 4], f32)
    ones_t = din("ones", [128, 1], f32)
    iota_t = din("iota512", [128, 512], f32)
    cntbc_t = din("cntbc", [128, 128], f32)
    loss_t = nc.dram_tensor("loss", [1, 1], f32, kind="ExternalOutput")

    # z split per int16-gather window so L1 gathers on window w wait only
    # for P1's writes to window w (overlaps P1 with the L1 gather pipeline)
    z_ws = [nc.dram_tensor(f"zarr{w}", [min(c.WIN, RT - w * c.WIN), 512], bf,
                           kind="Internal")
            for w in range(meta["NW1"])]

    with tile.TileContext(nc) as tc:
        with (
            tc.tile_pool(name="const", bufs=1) as cpool,
            tc.tile_pool(name="dram", bufs=1, space="DRAM") as dpool,
        ):
            z2own = [dpool.tile([c.OWNP, 128], bf, tag=f"z2own{d}", name=f"z2own{d}") for d in range(2)]
            z2full = [dpool.tile([c.NPAD, 128], bf, addr_space="Shared", tag=f"z2full{d}", name=f"z2full{d}")
                      for d in range(2)]
            ar_in = dpool.tile([128, 520], f32, tag="arin", name="arin")
            ar_out = dpool.tile([128, 520], f32, addr_space="Shared", tag="arout", name="arout")

            ident = cpool.tile([128, 128], bf)
            make_identity(nc, ident[:])
            w1sb = cpool.tile([128, 4 * 512], bf)
            for k in range(4):
                nc.sync.dma_start(out=w1sb[:, k * 512:(k + 1) * 512],
                                  in_=w1all[k * 128:(k + 1) * 128, :])
            w2sb = [cpool.tile([128, 128], bf, tag=f"w2_{d}", name=f"w2_{d}") for d in range(2)]
            tonsb = cpool.tile([128, 256], bf)
            b1sb = [cpool.tile([128, 256], f32, tag=f"b1_{d}", name=f"b1_{d}") for d in range(2)]
            dlsb = [cpool.tile([128, RT // 128], f32, tag=f"dl_{d}", name=f"dl_{d}") for d in range(2)]
            ddsb = [cpool.tile([128, c.NB], f32, tag=f"dd_{d}", name=f"dd_{d}") for d in range(2)]
            onesb = cpool.tile([128, 1], f32)
            iota128 = cpool.tile([128, 128], f32)
            nc.sync.dma_start(out=tonsb[:], in_=tonbc[:, :])
            nc.sync.dma_start(out=onesb[:], in_=ones_t[:, :])
            nc.sync.dma_start(out=iota128[:], in_=iota_t[:, 0:128])
            for d in range(2):
                nc.sync.dma_start(out=w2sb[d][:], in_=w2[d][:, :])
                nc.sync.dma_start(out=b1sb[d][:], in_=b1bc[d][:, :])
                nc.sync.dma_start(out=dlsb[d][:], in_=dloc[d][:, :])
                nc.sync.dma_start(out=ddsb[d][:], in_=ddst[d][:, :])

            # ================= P1: z = scaled([x1|x] @ W1-fused) ==========
            with (
                tc.tile_pool(name="xk", bufs=2) as xkp,
                tc.tile_pool(name="zsb", bufs=3) as zsp,
                tc.tile_pool(name="pz", bufs=2, space="PSUM") as pzp,
            ):
                for sec, (r0, rlen) in enumerate(((0, RU_PAD), (RU_PAD, RK_PAD))):
                    for off in range(0, rlen, c.NF):
                        nf = min(c.NF, rlen - off)
                        xk = xkp.tile([128, 4 * c.NF], bf, tag="xk", name="xk")
                        for k in range(4):
                            nc.sync.dma_start(
                                out=xk[:, k * c.NF:k * c.NF + nf],
                                in_=xT[k * 128:(k + 1) * 128, r0 + off:r0 + off + nf])
                        for j in range(nf // 128):
                            row = r0 + off + j * 128
                            rb = row // 128
                            if sec == 0:
                                ps = pzp.tile([128, 512], f32, tag="pz", name="pz")
                                for k in range(4):
                                    nc.tensor.matmul(
                                        out=ps[:],
                                        lhsT=xk[:, k * c.NF + j * 128:k * c.NF + (j + 1) * 128],
                                        rhs=w1sb[:, k * 512:(k + 1) * 512],
                                        start=(k == 0), stop=(k == 3))
                                zs = zsp.tile([128, 512], bf, tag="zs", name="zs")
                                nc.scalar.activation(
                                    out=zs[:, 0:256], in_=ps[:, 0:256],
                                    func=mybir.ActivationFunctionType.Copy,
                                    scale=dlsb[0][:, rb:rb + 1])
                                nc.scalar.activation(
                                    out=zs[:, 256:512], in_=ps[:, 256:512],
                                    func=mybir.ActivationFunctionType.Copy,
                                    scale=dlsb[1][:, rb:rb + 1])
                            else:
                                ps = pzp.tile([128, 512], f32, tag="pz", name="pz")
                                for k in range(4):
                                    nc.tensor.matmul(
                                        out=ps[:, 0:128],
                                        lhsT=xk[:, k * c.NF + j * 128:k * c.NF + (j + 1) * 128],
                                        rhs=w1sb[:, k * 512 + 128:k * 512 + 256],
                                        start=(k == 0), stop=(k == 3))
                                for k in range(4):
                                    nc.tensor.matmul(
                                        out=ps[:, 128:256],
                                        lhsT=xk[:, k * c.NF + j * 128:k * c.NF + (j + 1) * 128],
                                        rhs=w1sb[:, k * 512 + 384:k * 512 + 512],
                                        start=(k == 0), stop=(k == 3))
                                zs = zsp.tile([128, 512], bf, tag="zs", name="zs")
                                nc.vector.tensor_scalar(
                                    out=zs[:, 0:128], in0=tonsb[:, 0:128],
                                    scalar1=dlsb[0][:, rb:rb + 1], scalar2=None, op0=MUL)
                                nc.scalar.activation(
                                    out=zs[:, 128:256], in_=ps[:, 0:128],
                                    func=mybir.ActivationFunctionType.Copy,
                                    scale=dlsb[0][:, rb:rb + 1])
                                nc.vector.tensor_scalar(
                                    out=zs[:, 256:384], in0=tonsb[:, 128:256],
                                    scalar1=dlsb[1][:, rb:rb + 1], scalar2=None, op0=MUL)
                                nc.scalar.activation(
                                    out=zs[:, 384:512], in_=ps[:, 128:256],
                                    func=mybir.ActivationFunctionType.Copy,
                                    scale=dlsb[1][:, rb:rb + 1])
                            zw = row // c.WIN
                            zr = row - zw * c.WIN
                            nc.sync.dma_start(out=z_ws[zw][zr:zr + 128, :],
                                              in_=zs[:])

            # ================= helper: one aggregation layer ==============
            def agg_layer(layer, d, sch, s_in, i_in, src_t, src_cols, elem,
                          estep, poolps):
                NW, KT, ktoff = sch["NW"], sch["KT"], sch["ktoff"]
                wlen = lambda w: min(c.WIN, (RT if layer == 0 else c.NPAD) - w * c.WIN)
                with (
                    tc.tile_pool(name=f"g{layer}{d}", bufs=3) as gp,
                    tc.tile_pool(name=f"sI{layer}{d}", bufs=3) as sp,
                    tc.tile_pool(name=f"ix{layer}{d}", bufs=3) as ip,
                    tc.tile_pool(name=f"fin{layer}{d}", bufs=3) as fp,
                    tc.tile_pool(name=f"pp{layer}{d}", bufs=2) as ppp,
                    tc.tile_pool(name=f"agg{layer}{d}", bufs=2, space="PSUM") as ap,
                    tc.tile_pool(name=f"tr{layer}{d}", bufs=2, space="PSUM") as trp,
                ):
                    GB = sch["GB"]
                    for g, ops in enumerate(sch["groups"]):
                        blks = range(g * GB, min((g + 1) * GB, c.NB))
                        if not ops:
                            continue
                        gkt0 = ops[0][1]
                        gnkt = sum(nk for _, _, nk in ops)
                        st = sp.tile([128, gnkt * 128], bf, tag="s", name="s")
                        nc.sync.dma_start(
                            out=st[:], in_=s_in[:, gkt0 * 128:(gkt0 + gnkt) * 128])
                        it = ip.tile([128, gnkt * 8], i16, tag="i", name="i")
                        nc.sync.dma_start(
                            out=it[:], in_=i_in[:, gkt0 * 8:(gkt0 + gnkt) * 8])
                        gt = gp.tile([128, gnkt * elem], bf, tag="g", name="g")
                        optiles = {}
                        for w, base, nk in ops:
                            o = base - gkt0
                            nc.gpsimd.dma_gather(
                                gt[:, o * elem:(o + nk) * elem].rearrange(
                                    "p (k e) -> p k e", k=nk, e=elem),
                                src_t[w][0:wlen(w),
                                         src_cols[0]:src_cols[1]],
                                it[:, o * 8:(o + nk) * 8], nk * 128, nk * 128, elem,
                                elem_step=estep, single_packet=False)
                            optiles[w] = (gt, gkt0)
                        if layer == 0:
                            ptile = None
                        else:
                            ptile = ppp.tile([128, len(blks) * 128], f32, tag="pp", name="pp")
                            b0 = g * GB
                            nc.sync.dma_start(
                                out=ptile[:],
                                in_=ppool_t[:, b0 * 128:(b0 + len(blks)) * 128])
                        for b in blks:
                            nkb = int(KT[b].sum())
                            if nkb == 0:
                                continue
                            fw = 256 if layer == 0 else 128
                            ps = ap.tile([128, fw], f32, tag="a", name="a")
                            emitted = 0
                            for w in range(NW):
                                if KT[b, w] == 0:
                                    continue
                                gt, base = optiles[w]
                                for t in range(KT[b, w]):
                                    kt = ktoff[b, w] + t
                                    nc.tensor.matmul(
                                        out=ps[:],
                                        lhsT=st[:, (kt - gkt0) * 128:(kt - gkt0 + 1) * 128],
                                        rhs=gt[:, (kt - base) * elem:(kt - base + 1) * elem],
                                        start=(emitted == 0),
                                        stop=(emitted == nkb - 1))
                                    emitted += 1
                            # finalize: (ps * dinv_dst) + bias
                            bias = b1sb[d] if layer == 0 else b2sb[d]
                            nc.vector.scalar_tensor_tensor(
                                out=ps[:], in0=ps[:], scalar=ddsb[d][:, b:b + 1],
                                in1=bias[:, 0:fw], op0=MUL, op1=ADD)
                            if layer == 0:
                                h1 = fp.tile([128, 256], bf, tag="h1", name="h1")
                                nc.scalar.activation(
                                    out=h1[:], in_=ps[:],
                                    func=mybir.ActivationFunctionType.Relu)
                                trt = trp.tile([128, 256], bf, tag="t", name="t")
                                nc.tensor.transpose(
                                    out=trt[:, 0:128], in_=h1[:, 0:128], identity=ident[:])
                                nc.tensor.transpose(
                                    out=trt[:, 128:256], in_=h1[:, 128:256], identity=ident[:])
                                h1T = fp.tile([128, 256], bf, tag="h1T", name="h1T")
                                nc.vector.tensor_copy(out=h1T[:], in_=trt[:])
                                z2ps = trp.tile([128, 128], f32, tag="z2", name="z2")
                                nc.tensor.matmul(out=z2ps[:, 0:64],
                                                 lhsT=h1T[:, 0:128],
                                                 rhs=w2sb[d][:, 0:64],
                                                 start=True, stop=True)
                                nc.tensor.matmul(out=z2ps[:, 64:128],
                                                 lhsT=h1T[:, 128:256],
                                                 rhs=w2sb[d][:, 64:128],
                                                 start=True, stop=True)
                                z2sb = fp.tile([128, 128], bf, tag="z2sb", name="z2sb")
                                nc.scalar.activation(
                                    out=z2sb[:], in_=z2ps[:],
                                    func=mybir.ActivationFunctionType.Copy,
                                    scale=ddsb[d][:, b:b + 1])
                                nc.sync.dma_start(
                                    out=z2own[d][b * 128:(b + 1) * 128, :], in_=z2sb[:])
                            else:
                                hsb = fp.tile([128, 128], f32, tag="hsb", name="hsb")
                                nc.scalar.copy(out=hsb[:], in_=ps[:])
                                nc.tensor.matmul(
                                    out=poolps[:], lhsT=ptile[:, (b - g * GB) * 128:(b - g * GB + 1) * 128],
                                    rhs=hsb[:], start=(b == 0), stop=(b == c.NB - 1),
                                    skip_group_check=True)
                                nc.sync.dma_start(
                                    out=hown[d][b * 128:(b + 1) * 128, :], in_=hsb[:])

            # ===== L1 (both dirs); AllGather issued per dir so the dir-0
            # collective overlaps dir-1 aggregation =======================
            for d in range(2):
                agg_layer(0, d, meta["sched1"][d], s1[d], i1[d],
                          z_ws, (256 * d, 256 * d + 256), 256, 512, None)
                nc.gpsimd.collective_compute(
                    "AllGather", mybir.AluOpType.bypass,
                    replica_groups=[list(range(c.C))],
                    ins=[z2own[d].opt()], outs=[z2full[d].opt()])

            # ========== pool: dense value-S matmuls over z2own ===========
            arsb = cpool.tile([128, 520], f32)
            nc.vector.memset(arsb[:], 0.0)
            with (
                tc.tile_pool(name="plz", bufs=3) as zp,
                tc.tile_pool(name="plS", bufs=2) as pwp,
                tc.tile_pool(name="plps", bufs=1, space="PSUM") as plp,
            ):
                poolps = [plp.tile([128, 256], f32, tag=f"pl{d}", name=f"pl{d}")
                          for d in range(2)]
                PB = 8
                for d in range(2):
                    for b0 in range(0, c.NB, PB):
                        nb = min(PB, c.NB - b0)
                        pst = pwp.tile([128, nb * 128], bf, tag="ps", name="ps")
                        nc.sync.dma_start(
                            out=pst[:], in_=pools_t[d][:, b0 * 128:(b0 + nb) * 128])
                        for j in range(nb):
                            b = b0 + j
                            zt = zp.tile([128, 128], bf, tag="z", name="z")
                            nc.sync.dma_start(
                                out=zt[:], in_=z2own[d][b * 128:(b + 1) * 128, :])
                            nc.tensor.matmul(
                                out=poolps[d][0:64, 0:128], lhsT=zt[:, 0:64],
                                rhs=pst[:, j * 128:(j + 1) * 128],
                                start=(b == 0), stop=(b == c.NB - 1),
                                skip_group_check=True)
                            nc.tensor.matmul(
                                out=poolps[d][0:64, 128:256], lhsT=zt[:, 64:128],
                                rhs=pst[:, j * 128:(j + 1) * 128],
                                start=(b == 0), stop=(b == c.NB - 1),
                                skip_group_check=True)
                for d in range(2):
                    nc.vector.tensor_copy(out=arsb[0:64, d * 256:(d + 1) * 256],
                                          in_=poolps[d][0:64, :])

            # ========== masked-node aggregation + cosine (feat-major) ====
            macc = cpool.tile([1, 1], f32)
            nc.vector.memset(macc[:], 0.0)
            with (
                tc.tile_pool(name="mix", bufs=1) as mip,
                tc.tile_pool(name="mgt", bufs=3) as mgp,
                tc.tile_pool(name="msg", bufs=3) as msp,
                tc.tile_pool(name="mfin", bufs=2) as mfp,
                tc.tile_pool(name="mscr", bufs=2) as sc2,
                tc.tile_pool(name="mps", bufs=2, space="PSUM") as mpp,
                tc.tile_pool(name="dps", bufs=2, space="PSUM") as dpp,
            ):
                iotasb = mip.tile([128, 512], f32, tag="iota", name="iota")
                nc.sync.dma_start(out=iotasb[:], in_=iota_t[:, :])
                mcvsb = mip.tile([1, MBG * 512], f32, tag="mcv", name="mcv")
                nc.sync.dma_start(out=mcvsb[:], in_=mcv_t[:, :])
                b2t = mip.tile([64, 4], f32, tag="b2t", name="b2t")
                nc.sync.dma_start(out=b2t[:], in_=b2col_t[:, :])
                mit, mdc, mwv, sit, sdc, swv = [], [], [], [], [], []
                for d in range(2):
                    TOT = meta["msched"][d]["TOT"]
                    STOT = meta["msched"][d]["STOT"]
                    t1 = mip.tile([128, TOT * 8], i16, tag=f"mi{d}", name=f"mi{d}")
                    nc.sync.dma_start(out=t1[:], in_=mi_t[d][:, :])
                    t2 = mip.tile([128, TOT], f32, tag=f"mdc{d}", name=f"mdc{d}")
                    nc.sync.dma_start(out=t2[:], in_=mdc_t[d][:, :])
                    t3 = mip.tile([128, TOT], f32, tag=f"mwv{d}", name=f"mwv{d}")
                    nc.sync.dma_start(out=t3[:], in_=mwv_t[d][:, :])
                    t4 = mip.tile([128, STOT * 8], i16, tag=f"si{d}", name=f"si{d}")
                    nc.sync.dma_start(out=t4[:], in_=si_t[d][:, :])
                    t5 = mip.tile([128, STOT], f32, tag=f"sdc{d}", name=f"sdc{d}")
                    nc.sync.dma_start(out=t5[:], in_=sdc_t[d][:, :])
                    t6 = mip.tile([128, STOT], f32, tag=f"swv{d}", name=f"swv{d}")
                    nc.sync.dma_start(out=t6[:], in_=swv_t[d][:, :])
                    mit.append(t1); mdc.append(t2); mwv.append(t3)
                    sit.append(t4); sdc.append(t5); swv.append(t6)

                for bg in range(MBG):
                    fins = {}
                    for d in range(2):
                        sch = meta["msched"][d]
                        KT, ktoff = sch["KT"], sch["ktoff"]
                        nkb = int(KT[bg].sum()) + int(sch["SKT"][bg])
                        onf = mfp.tile([128, 512], f32, tag=f"on{d}", name=f"on{d}")
                        tgf = mfp.tile([128, 512], f32, tag=f"tg{d}", name=f"tg{d}")
                        if nkb == 0:
                            nc.vector.memset(onf[:], 0.0)
                            nc.vector.memset(tgf[:], 0.0)
                            fins[d] = (onf, tgf)
                            continue
                        pson = mpp.tile([128, 512], f32, tag="pon", name="pon")
                        pstg = mpp.tile([128, 512], f32, tag="ptg", name="ptg")
                        emitted = 0

                        def mm_tiles(gt, nk, o, dct, wvt):
                            nonlocal emitted
                            for t in range(nk):
                                st = msp.tile([128, 512], bf, tag="ms", name="ms")
                                nc.vector.tensor_scalar(
                                    out=st[:], in0=iotasb[:],
                                    scalar1=dct[:, o + t:o + t + 1],
                                    scalar2=wvt[:, o + t:o + t + 1],
                                    op0=ISEQ, op1=MUL)
                                nc.tensor.matmul(
                                    out=pson[0:64, :],
                                    lhsT=gt[:, t * 128:t * 128 + 64],
                                    rhs=st[:], start=(emitted == 0),
                                    stop=(emitted == nkb - 1),
                                    skip_group_check=True)
                                nc.tensor.matmul(
                                    out=pstg[0:64, :],
                                    lhsT=gt[:, t * 128 + 64:t * 128 + 128],
                                    rhs=st[:], start=(emitted == 0),
                                    stop=(emitted == nkb - 1),
                                    skip_group_check=True)
                                emitted += 1

                        for w in range(NW2):
                            nk = int(KT[bg, w])
                            if nk == 0:
                                continue
                            o = int(ktoff[bg, w])
                            wl = min(c.WIN, c.NPAD - w * c.WIN)
                            gt = mgp.tile([128, nk * 128], bf, tag="mg", name="mg")
                            nc.gpsimd.dma_gather(
                                gt[:].rearrange("p (k e) -> p k e", k=nk, e=128),
                                z2full[d][w * c.WIN:w * c.WIN + wl, :],
                                mit[d][:, o * 8:(o + nk) * 8], nk * 128, nk * 128,
                                128, elem_step=None, single_packet=False)
                            mm_tiles(gt, nk, o, mdc[d], mwv[d])
                        snk = int(sch["SKT"][bg])
                        if snk:
                            so = int(sch["skoff"][bg])
                            gt = mgp.tile([128, snk * 128], bf, tag="sg", name="sg")
                            nc.gpsimd.dma_gather(
                                gt[:].rearrange("p (k e) -> p k e", k=snk, e=128),
                                z2own[d][0:c.OWNP, :],
                                sit[d][:, so * 8:(so + snk) * 8], snk * 128,
                                snk * 128, 128, elem_step=None,
                                single_packet=False)
                            mm_tiles(gt, snk, so, sdc[d], swv[d])
                        nc.vector.tensor_scalar(
                            out=onf[0:64, :], in0=pson[0:64, :],
                            scalar1=b2t[0:64, 2 * d:2 * d + 1], scalar2=None,
                            op0=ADD)
                        nc.vector.tensor_scalar(
                            out=tgf[0:64, :], in0=pstg[0:64, :],
                            scalar1=b2t[0:64, 2 * d + 1:2 * d + 2], scalar2=None,
                            op0=ADD)
                        fins[d] = (onf, tgf)

                    # dot / |on|^2 / |tgt|^2 summed over feat (64) and dirs
                    sums = []
                    for qi, pick in enumerate(((0, 1), (0, 0), (1, 1))):
                        qp = dpp.tile([1, 512], f32, tag="qp", name="qp")
                        for d in range(2):
                            pa = fins[d][pick[0]]
                            pb = fins[d][pick[1]]
                            pr = sc2.tile([64, 512], f32, tag=f"pr{d}", name=f"pr{d}")
                            nc.vector.tensor_tensor(
                                out=pr[:], in0=pa[0:64, :], in1=pb[0:64, :], op=MUL)
                            nc.tensor.matmul(
                                out=qp[:], lhsT=onesb[0:64, 0:1], rhs=pr[:],
                                start=(d == 0), stop=(d == 1),
                                skip_group_check=True)
                        sq = sc2.tile([1, 512], f32, tag=f"sq{qi}", name=f"sq{qi}")
                        nc.vector.tensor_copy(out=sq[:], in_=qp[:])
                        sums.append(sq)
                    sdot, sn1, sn2 = sums

                    def rguard(n, tag):
                        r = sc2.tile([1, 512], f32, tag=tag, name=tag)
                        nc.scalar.sqrt(out=r[:], in_=n[:])
                        nc.vector.tensor_scalar_max(out=r[:], in0=r[:], scalar1=1e-12)
                        nc.vector.reciprocal(out=r[:], in_=r[:])
                        return r

                    r1 = rguard(sn1, "r1")
                    r2 = rguard(sn2, "r2")
                    cv = mcvsb[:, bg * 512:(bg + 1) * 512]
                    tt = sc2.tile([1, 512], f32, tag="tt", name="tt")
                    nc.vector.tensor_tensor(out=tt[:], in0=sdot[:], in1=r1[:], op=MUL)
                    nc.vector.tensor_tensor(out=tt[:], in0=tt[:], in1=r2[:], op=MUL)
                    nc.vector.tensor_tensor(out=tt[:], in0=tt[:], in1=cv, op=MUL)
                    scr = sc2.tile([1, 512], f32, tag="scr", name="scr")
                    ts = sc2.tile([1, 1], f32, tag="ts", name="ts")
                    nc.vector.scalar_tensor_tensor(
                        out=scr[:], in0=tt[:], scalar=-1.0, in1=cv,
                        op0=MUL, op1=ADD, accum_out=ts[:])
                    nc.vector.tensor_tensor(out=macc[:], in0=macc[:], in1=ts[:],
                                            op=ADD)

            # ========== AllReduce (pools + mask partial) =================
            nc.vector.tensor_copy(out=arsb[0:1, 512:513], in_=macc[:])
            nc.sync.dma_start(out=ar_in[:, :], in_=arsb[:])
            nc.gpsimd.collective_compute(
                "AllReduce", mybir.AluOpType.add,
                replica_groups=[list(range(c.C))],
                ins=[ar_in.opt()], outs=[ar_out.opt()])

            # ========== pooled cosine + final loss =======================
            with (
                tc.tile_pool(name="fin2", bufs=2) as f2,
                tc.tile_pool(name="fps", bufs=2, space="PSUM") as fpp,
            ):
                ar2 = f2.tile([128, 520], f32, tag="ar2", name="ar2")
                nc.sync.dma_start(out=ar2[:], in_=ar_out[:, :])
                cntsb = f2.tile([128, 128], f32, tag="cnt", name="cnt")
                nc.sync.dma_start(out=cntsb[:], in_=cntbc_t[:, :])
                b2t = f2.tile([64, 4], f32, tag="b2tf", name="b2tf")
                nc.sync.dma_start(out=b2t[:], in_=b2col_t[:, :])
                # pool slices per (dir, head) + cnt*b2 bias
                pf = {}
                for d in range(2):
                    for h in range(2):
                        po = f2.tile([64, 128], f32, tag=f"po{d}{h}", name=f"po{d}{h}")
                        nc.vector.scalar_tensor_tensor(
                            out=po[:], in0=cntsb[0:64, :],
                            scalar=b2t[0:64, 2 * d + h:2 * d + h + 1],
                            in1=ar2[0:64, d * 256 + h * 128:d * 256 + (h + 1) * 128],
                            op0=MUL, op1=ADD)
                        pf[(d, h)] = po
                gsums = []
                for qi, pick in enumerate(((0, 1), (0, 0), (1, 1))):
                    qp = fpp.tile([1, 128], f32, tag="gqp", name="gqp")
                    for d in range(2):
                        pr = f2.tile([64, 128], f32, tag=f"gpr{d}", name=f"gpr{d}")
                        nc.vector.tensor_tensor(
                            out=pr[:], in0=pf[(d, pick[0])][:],
                            in1=pf[(d, pick[1])][:], op=MUL)
                        nc.tensor.matmul(
                            out=qp[:], lhsT=onesb[0:64, 0:1], rhs=pr[:],
                            start=(d == 0), stop=(d == 1), skip_group_check=True)
                    sq = f2.tile([1, 128], f32, tag=f"gsq{qi}", name=f"gsq{qi}")
                    nc.vector.tensor_copy(out=sq[:], in_=qp[:])
                    gsums.append(sq)
                gdot, gn1, gn2 = gsums

                def rguard2(n, tag):
                    r = f2.tile([1, 128], f32, tag=tag, name=tag)
                    nc.scalar.sqrt(out=r[:], in_=n[:])
                    nc.vector.tensor_scalar_max(out=r[:], in0=r[:], scalar1=1e-12)
                    nc.vector.reciprocal(out=r[:], in_=r[:])
                    return r

                g1 = rguard2(gn1, "g1")
                g2 = rguard2(gn2, "g2")
                cosg = f2.tile([1, 128], f32, tag="cosg", name="cosg")
                nc.vector.tensor_tensor(out=cosg[:], in0=gdot[:], in1=g1[:], op=MUL)
                nc.vector.tensor_tensor(out=cosg[:], in0=cosg[:], in1=g2[:], op=MUL)
                onesrow = f2.tile([1, 128], f32, tag="onesr", name="onesr")
                nc.vector.memset(onesrow[:], 1.0)
                gterm = f2.tile([1, 128], f32, tag="gterm", name="gterm")
                gs = f2.tile([1, 1], f32, tag="gs", name="gs")
                nc.vector.scalar_tensor_tensor(
                    out=gterm[:], in0=cosg[:], scalar=-1.0, in1=onesrow[:],
                    op0=MUL, op1=ADD, accum_out=gs[:])
                l1t = f2.tile([1, 1], f32, tag="l1", name="l1")
                nc.scalar.activation(out=l1t[:], in_=gs[:],
                                     func=mybir.ActivationFunctionType.Copy,
                                     scale=1.0 / c.G)
                l2t = f2.tile([1, 1], f32, tag="l2", name="l2")
                nc.scalar.activation(out=l2t[:], in_=ar2[0:1, 512:513],
                                     func=mybir.ActivationFunctionType.Copy,
                                     scale=1.0 / c.M)
                nc.vector.tensor_tensor(out=l1t[:], in0=l1t[:], in1=l2t[:], op=ADD)
                nc.sync.dma_start(out=loss_t[:, :], in_=l1t[:])

    return nc


# ---------------------------------------------------------------- entry

LAST_RESULT = None


def _install_trace_hook():
    """The agent image's antenv lacks axon_hooks; synthesize it from
    trn_boot's ctypes NTFF hook so trace=True works under axon."""
    import types
    try:
        from antenv import axon_hooks  # noqa: F401
        return
    except ImportError:
        pass
    try:
        import antenv
        import trn_agent_boot.trn_boot as tb
        hook = tb._ntff_profile_via_ctypes("/opt/axon/libaxon_pjrt.so")
        mod = types.ModuleType("antenv.axon_hooks")
        mod.get_axon_ntff_profile_hook = lambda: hook
        mod.set_axon_ntff_profile_hook = lambda h: None
        sys.modules["antenv.axon_hooks"] = mod
        antenv.axon_hooks = mod
    except Exception as e:
        print(f"[kernel] trace hook install failed: {e}", file=sys.stderr)


def kernel(_trace=False, **inputs):
    global LAST_RESULT
    import time
    from concourse import bass_utils
    if _trace:
        _install_trace_hook()
    cfg = FULL
    t0 = time.monotonic()
    meta, in_maps = host_prep(cfg, inputs)
    t1 = time.monotonic()
    nc = build_program(cfg, meta)
    t2 = time.monotonic()
    nc.compile()
    t3 = time.monotonic()
    res = bass_utils.run_bass_kernel_spmd(
        nc, in_maps, core_ids=list(range(cfg.C)),
        trace=_trace, trace_cores=[0] if _trace else None)
    t4 = time.monotonic()
    print(f"[kernel] prep {t1-t0:.1f}s build {t2-t1:.1f}s "
          f"compile {t3-t2:.1f}s run {t4-t3:.1f}s", file=sys.stderr)
    LAST_RESULT = res
    return np.float32(res.results[0]["loss"][0, 0])

